# revision 1
# baseline (speedup 1.0000x reference)
"""Trainium2 Bass kernel for nn_GAT_LSTM: 3-layer GATv2 stack + LSTM + FC head.

Sharding (8 NeuronCores):
  Launch A: data-parallel over the 40 (B,T) graphs -> 5 graphs/core. GATv2
            edges are sorted by dst into 128-node blocks; segment softmax and
            aggregation run as per-chunk one-hot matmuls on the PE; xl[src]
            is fetched with indirect DMA gathers.
  Launch B: LSTM w_ih [1024,64000] column-sharded 8-way; each core computes
            partial gate pre-activations for all 40 graphs.
  Launch C: partial sums reduced on-device, tiny LSTM scan + FC head.
Host work between launches is indexing/reshaping only.
"""
import sys

for _p in ("/opt/trn_rl_repo", "/root/.axon_site/_ro/trn_rl_repo"):
    if _p not in sys.path:
        sys.path.insert(0, _p)

import numpy as np

import concourse.bass as bass
import concourse.bacc as bacc
import concourse.mybir as mybir
import concourse.tile as tile
from concourse import bass_utils
from concourse.masks import make_identity

F32 = mybir.dt.float32
I32 = mybir.dt.int32
AF = mybir.ActivationFunctionType
OP = mybir.AluOpType

P = 128
N = 2000
NPAD = 2048
NBLK = 16
B, T = 4, 10
G = B * T          # 40 graphs
NCORES = 8
GL = G // NCORES   # 5 graphs per core
E0 = 16000
EFULL = E0 + N     # with self loops
# layers: (F_in, H, C) ; F_out = H*C
LAYERS = [(8, 4, 32), (128, 4, 32), (128, 4, 8)]
HID = 256
GATE = 4 * HID     # 1024
EMB = N * 32       # 64000
KSL = EMB // NCORES          # 8000 w_ih columns per core
KPAD = ((KSL + 127) // 128) * 128  # 8064

_cache = {}


# ----------------------------------------------------------------------------
# host-side graph preprocessing (pure indexing + tiny edge-static math)
# ----------------------------------------------------------------------------
def prep_graph(edge_index, edge_attr, weights):
    src = np.concatenate([edge_index[0], np.arange(N, dtype=np.int64)])
    dst = np.concatenate([edge_index[1], np.arange(N, dtype=np.int64)])
    ea = np.concatenate(
        [edge_attr, np.broadcast_to(edge_attr.mean(0), (N, 2))], axis=0
    ).astype(np.float32)

    order = np.argsort(dst, kind="stable")
    src_s, dst_s, ea_s = src[order], dst[order], ea[order]

    # per 128-dst-node block, pad edge count to a multiple of 128
    blk_of = dst_s // P
    chunks_per_blk = []
    src_pad, ldst_pad, valid_pad, ea_pad = [], [], [], []
    for b in range(NBLK):
        sel = blk_of == b
        e_src = src_s[sel]
        e_ldst = dst_s[sel] - b * P
        e_ea = ea_s[sel]
        ne = len(e_src)
        nch = max(1, (ne + P - 1) // P)
        pad = nch * P - ne
        src_pad.append(np.concatenate([e_src, np.zeros(pad, np.int64)]))
        ldst_pad.append(np.concatenate([e_ldst, np.zeros(pad, np.int64)]))
        valid_pad.append(np.concatenate([np.ones(ne, bool), np.zeros(pad, bool)]))
        ea_pad.append(np.concatenate([e_ea, np.zeros((pad, 2), np.float32)]))
        chunks_per_blk.append(nch)

    src_all = np.concatenate(src_pad)      # [NCH*128]
    ldst_all = np.concatenate(ldst_pad)
    valid_all = np.concatenate(valid_pad)
    ea_all = np.concatenate(ea_pad)
    nch_total = sum(chunks_per_blk)

    idx = src_all.reshape(nch_total, P).T.astype(np.int32).copy()      # [128, NCH]
    # one-hot scatter matrices, zeroed rows for pad edges
    pen = np.zeros((nch_total, P, P), np.float32)                      # [e, n]
    jj = np.repeat(np.arange(nch_total), P)
    ee_pos = np.tile(np.arange(P), nch_total)
    pen[jj[valid_all], ee_pos[valid_all], ldst_all[valid_all]] = 1.0
    pne = np.ascontiguousarray(pen.transpose(0, 2, 1))                 # [n, e]

    # per-layer edge-feature terms ee = ea @ We  (edge-static, shared by all
    # graphs/timesteps; computed host-side once)
    ees = []
    for li, key in enumerate(("w_e0", "w_e1", "w_e2")):
        we = np.asarray(weights[key], np.float32)
        ee = (ea_all @ we).astype(np.float32)      # [NCH*128, F_out]
        ees.append(ee.reshape(nch_total, P, ee.shape[1]))
    return dict(
        chunks_per_blk=chunks_per_blk, nch_total=nch_total,
        idx=idx, pen=pen, pne=pne, ees=ees,
    )


def _bcast_const(vec, reps):
    """[F] -> [128, reps*F] partition-broadcast constant."""
    t = np.tile(np.asarray(vec, np.float32).reshape(-1), reps)
    return np.ascontiguousarray(np.broadcast_to(t, (P, t.size)))


# ----------------------------------------------------------------------------
# Launch A: GAT stack, 5 graphs per core
# ----------------------------------------------------------------------------
def build_gat(chunks_per_blk):
    nch_total = sum(chunks_per_blk)
    nc = bacc.Bacc("TRN2", target_bir_lowering=False, debug=False,
                   enable_asserts=False, num_devices=NCORES)
    # inputs
    xT_d = nc.dram_tensor("xT", [GL, 8, NPAD], F32, kind="ExternalInput")
    w01_d = nc.dram_tensor("w01", [8, 256], F32, kind="ExternalInput")
    w11_d = nc.dram_tensor("w11", [128, 256], F32, kind="ExternalInput")
    w21_d = nc.dram_tensor("w21", [128, 64], F32, kind="ExternalInput")
    ee_d = [nc.dram_tensor(f"ee{l}", [nch_total, P, LAYERS[l][1] * LAYERS[l][2]],
                           F32, kind="ExternalInput") for l in range(3)]
    pen_d = nc.dram_tensor("pen", [nch_total, P, P], F32, kind="ExternalInput")
    pne_d = nc.dram_tensor("pne", [nch_total, P, P], F32, kind="ExternalInput")
    idx_d = nc.dram_tensor("idx", [P, nch_total], I32, kind="ExternalInput")
    attb_d = [nc.dram_tensor(f"attb{l}", [P, GL * LAYERS[l][1] * LAYERS[l][2]],
                             F32, kind="ExternalInput") for l in range(3)]
    biasb_d = [nc.dram_tensor(f"biasb{l}", [P, GL * LAYERS[l][1] * LAYERS[l][2]],
                              F32, kind="ExternalInput") for l in range(3)]
    emb_d = nc.dram_tensor("emb", [GL, EMB], F32, kind="ExternalOutput")

    ws_d = [w01_d, w11_d, w21_d]

    with tile.TileContext(nc) as tc:
        with (
            tc.tile_pool(name="const", bufs=1) as cp,
            tc.tile_pool(name="node", bufs=1) as npool,
            tc.tile_pool(name="work", bufs=6) as wp,
            tc.tile_pool(name="stream", bufs=2) as strm,
            tc.tile_pool(name="psx", bufs=1, space="PSUM") as psx,
            tc.tile_pool(name="psacc", bufs=1, space="PSUM") as psacc,
            tc.tile_pool(name="pst", bufs=1, space="PSUM") as pst,
            tc.tile_pool(name="dram", bufs=1, space="DRAM") as dp,
        ):
            ident = cp.tile([P, P], F32)
            make_identity(nc, ident[:])
            idx_t = cp.tile([P, nch_total], I32)
            nc.sync.dma_start(out=idx_t[:], in_=idx_d[:, :])
            w_t = [cp.tile([8, 256], F32, tag="w0", name="w0t"),
                   cp.tile([128, 256], F32, tag="w1", name="w1t"),
                   cp.tile([128, 64], F32, tag="w2", name="w2t")]
            attb_t, biasb_t = [], []
            for l in range(3):
                nc.sync.dma_start(out=w_t[l][:], in_=ws_d[l][:, :])
                fo = LAYERS[l][1] * LAYERS[l][2]
                at = cp.tile([P, GL * fo], F32, tag=f"attb{l}", name=f"attb{l}t")
                bt = cp.tile([P, GL * fo], F32, tag=f"biasb{l}", name=f"biasb{l}t")
                nc.sync.dma_start(out=at[:], in_=attb_d[l][:, :])
                nc.sync.dma_start(out=bt[:], in_=biasb_d[l][:, :])
                attb_t.append(at)
                biasb_t.append(bt)

            # DRAM scratch: xl gather tables + inter-layer h
            xl_dram01 = dp.tile([NPAD, GL * 128], F32)
            xl_dram2 = dp.tile([NPAD, GL * 32], F32)
            h_dram = dp.tile([NPAD, GL, 128], F32)

            for l, (fin, hh, cc) in enumerate(LAYERS):
                fo = hh * cc
                gfo = GL * fo
                xl_dram = xl_dram2 if l == 2 else xl_dram01
                # ---- transform phase: xl/xr = h @ [Wl|Wr] ----
                xr_blocks = []
                for b in range(NBLK):
                    xr_b = npool.tile([P, GL, fo], F32, tag=f"xr{b}", name=f"xr{b}")
                    for g in range(GL):
                        if l == 0:
                            hx = wp.tile([8, P], F32, tag="hx0", name="hx0")
                            nc.sync.dma_start(
                                out=hx[:], in_=xT_d[g, :, b * P:(b + 1) * P])
                            lhsT = hx[:]
                        else:
                            hx = wp.tile([P, P], F32, tag="hx", name="hx")
                            nc.sync.dma_start(
                                out=hx[:], in_=h_dram[b * P:(b + 1) * P, g, :])
                            tr_ps = pst.tile([P, P], F32, tag="tr", name="tr")
                            nc.tensor.transpose(
                                out=tr_ps[:, :], in_=hx[:], identity=ident[:])
                            hT = wp.tile([P, P], F32, tag="hT", name="hT")
                            nc.scalar.activation(out=hT[:], in_=tr_ps[:, :], func=AF.Copy)
                            lhsT = hT[:]
                        xx_ps = pst.tile([P, 2 * fo], F32, tag="xx", name="xx")
                        nc.tensor.matmul(out=xx_ps[:, :], lhsT=lhsT,
                                         rhs=w_t[l][:fin, :2 * fo],
                                         start=True, stop=True)
                        nc.vector.tensor_copy(out=xr_b[:, g, :], in_=xx_ps[:, fo:2 * fo])
                        xl_st = wp.tile([P, fo], F32, tag="xl_st", name="xl_st")
                        nc.vector.tensor_copy(out=xl_st[:], in_=xx_ps[:, :fo])
                        nc.sync.dma_start(
                            out=xl_dram[:].rearrange("n (g f) -> n g f", g=GL)[
                                b * P:(b + 1) * P, g, :],
                            in_=xl_st[:])
                    xr_blocks.append(xr_b)

                # ---- edge phase ----
                j0 = 0
                for b in range(NBLK):
                    nch = chunks_per_blk[b]
                    ee_t = strm.tile([P, nch, fo], F32, tag="ee", name="ee")
                    nc.sync.dma_start(
                        out=ee_t[:],
                        in_=ee_d[l][j0:j0 + nch].rearrange("j p f -> p j f"))
                    pen_t = strm.tile([P, nch, P], F32, tag="pen", name="pen_t")
                    nc.sync.dma_start(
                        out=pen_t[:],
                        in_=pen_d[j0:j0 + nch].rearrange("j e n -> e j n"))
                    pne_t = strm.tile([P, nch, P], F32, tag="pne", name="pne_t")
                    nc.sync.dma_start(
                        out=pne_t[:],
                        in_=pne_d[j0:j0 + nch].rearrange("j n e -> n j e"))

                    ps_out = psacc.tile([P, gfo], F32, tag="pso", name="pso")
                    ps_den = psacc.tile([P, GL * hh], F32, tag="psd", name="psd")
                    for j in range(nch):
                        jg = j0 + j
                        # gather xl[src] rows for this chunk (all graphs)
                        g_t = wp.tile([P, gfo], F32, tag="g", name="g_t")
                        nc.gpsimd.indirect_dma_start(
                            out=g_t[:], out_offset=None,
                            in_=xl_dram[:, :],
                            in_offset=bass.IndirectOffsetOnAxis(
                                ap=idx_t[:, jg:jg + 1], axis=0))
                        # xr[dst] via one-hot matmul
                        ps_x = psx.tile([P, gfo], F32, tag="psx", name="ps_x")
                        for s0 in range(0, gfo, 512):
                            s1 = min(s0 + 512, gfo)
                            nc.tensor.matmul(out=ps_x[:, s0:s1],
                                             lhsT=pne_t[:, j, :],
                                             rhs=xr_blocks[b][:].rearrange(
                                                 "p g f -> p (g f)")[:, s0:s1],
                                             start=True, stop=True)
                        v_t = wp.tile([P, gfo], F32, tag="v", name="v_t")
                        nc.vector.tensor_tensor(out=v_t[:], in0=ps_x[:, :],
                                                in1=g_t[:], op=OP.add)
                        nc.vector.tensor_tensor(
                            out=v_t[:].rearrange("p (g f) -> p g f", g=GL),
                            in0=v_t[:].rearrange("p (g f) -> p g f", g=GL),
                            in1=ee_t[:, j:j + 1, :].to_broadcast([P, GL, fo]),
                            op=OP.add)
                        m_t = wp.tile([P, gfo], F32, tag="m", name="m_t")
                        nc.scalar.activation(out=m_t[:], in_=v_t[:],
                                             func=AF.Lrelu, alpha=0.2)
                        nc.vector.tensor_tensor(out=m_t[:], in0=m_t[:],
                                                in1=attb_t[l][:], op=OP.mult)
                        logit_t = wp.tile([P, GL * hh], F32, tag="logit", name="logit_t")
                        nc.vector.tensor_reduce(
                            out=logit_t[:],
                            in_=m_t[:].rearrange("p (t c) -> p t c", c=cc),
                            axis=mybir.AxisListType.X, op=OP.add)
                        p_t = wp.tile([P, GL * hh], F32, tag="p", name="p_t")
                        nc.scalar.activation(out=p_t[:], in_=logit_t[:], func=AF.Exp)
                        nc.tensor.matmul(out=ps_den[:, :], lhsT=pen_t[:, j, :],
                                         rhs=p_t[:], start=(j == 0),
                                         stop=(j == nch - 1))
                        pxl_t = wp.tile([P, gfo], F32, tag="pxl", name="pxl_t")
                        nc.vector.tensor_tensor(
                            out=pxl_t[:].rearrange("p (t c) -> p t c", c=cc),
                            in0=g_t[:].rearrange("p (t c) -> p t c", c=cc),
                            in1=p_t[:].rearrange("p (t c) -> p t c", c=1)
                                 .to_broadcast([P, GL * hh, cc]),
                            op=OP.mult)
                        for s0 in range(0, gfo, 512):
                            s1 = min(s0 + 512, gfo)
                            nc.tensor.matmul(out=ps_out[:, s0:s1],
                                             lhsT=pen_t[:, j, :],
                                             rhs=pxl_t[:, s0:s1],
                                             start=(j == 0), stop=(j == nch - 1))
                    # block tail: out = relu(ps_out * (1/(den+eps))[dst-node] + bias)
                    den_t = wp.tile([P, GL * hh], F32, tag="den", name="den_t")
                    nc.vector.tensor_scalar_add(out=den_t[:], in0=ps_den[:, :],
                                                scalar1=1e-16)
                    rec_t = wp.tile([P, GL * hh], F32, tag="rec", name="rec_t")
                    nc.vector.reciprocal(out=rec_t[:], in_=den_t[:])
                    rece_t = wp.tile([P, gfo], F32, tag="rece", name="rece_t")
                    nc.vector.tensor_copy(
                        out=rece_t[:].rearrange("p (t c) -> p t c", c=cc),
                        in_=rec_t[:].rearrange("p (t c) -> p t c", c=1)
                            .to_broadcast([P, GL * hh, cc]))
                    o_t = wp.tile([P, GL, fo], F32, tag="o_t", name="o_t")
                    nc.vector.tensor_tensor(
                        out=o_t[:].rearrange("p g f -> p (g f)"),
                        in0=ps_out[:, :], in1=rece_t[:], op=OP.mult)
                    nc.vector.tensor_tensor(
                        out=o_t[:].rearrange("p g f -> p (g f)"),
                        in0=o_t[:].rearrange("p g f -> p (g f)"),
                        in1=biasb_t[l][:], op=OP.add)
                    nc.scalar.activation(out=o_t[:].rearrange("p g f -> p (g f)"),
                                         in_=o_t[:].rearrange("p g f -> p (g f)"),
                                         func=AF.Relu)
                    if l < 2:
                        nc.sync.dma_start(
                            out=h_dram[b * P:(b + 1) * P, :, :fo], in_=o_t[:])
                    else:
                        rows = min(P, N - b * P)
                        nc.sync.dma_start(
                            out=emb_d[:, :].rearrange("g (n c) -> g n c", c=32)[
                                :, b * P:b * P + rows, :].rearrange("g p c -> p g c"),
                            in_=o_t[:rows, :, :])
                    j0 += nch
    nc.compile()
    return nc


# ----------------------------------------------------------------------------
# Launch B: partial LSTM input-gate products (w_ih column shard)
# ----------------------------------------------------------------------------
def build_gates():
    nc = bacc.Bacc("TRN2", target_bir_lowering=False, debug=False,
                   enable_asserts=False, num_devices=NCORES)
    embT_d = nc.dram_tensor("embT", [KPAD, G], F32, kind="ExternalInput")
    wT_d = nc.dram_tensor("wT", [KPAD, GATE], F32, kind="ExternalInput")
    part_d = nc.dram_tensor("part", [G, GATE], F32, kind="ExternalOutput")
    KCH = KPAD // P
    with tile.TileContext(nc) as tc:
        with (
            tc.tile_pool(name="sb", bufs=1) as sp,
            tc.tile_pool(name="wstream", bufs=4) as wsp,
            tc.tile_pool(name="ps", bufs=1, space="PSUM") as pp,
        ):
            embT_t = sp.tile([P, KCH, G], F32)
            nc.sync.dma_start(out=embT_t[:],
                              in_=embT_d[:, :].rearrange("(k p) g -> p k g", p=P))
            ps = pp.tile([G, GATE], F32)
            for k in range(KCH):
                w_t = wsp.tile([P, GATE], F32, tag="w")
                nc.sync.dma_start(out=w_t[:], in_=wT_d[k * P:(k + 1) * P, :])
                for s0 in range(0, GATE, 512):
                    nc.tensor.matmul(out=ps[:, s0:s0 + 512],
                                     lhsT=embT_t[:, k, :],
                                     rhs=w_t[:, s0:s0 + 512],
                                     start=(k == 0), stop=(k == KCH - 1))
            out_t = sp.tile([G, GATE], F32)
            nc.vector.tensor_copy(out=out_t[:], in_=ps[:, :])
            nc.sync.dma_start(out=part_d[:, :], in_=out_t[:])
    nc.compile()
    return nc


# ----------------------------------------------------------------------------
# Launch C: reduce partials + LSTM scan + FC head
# ----------------------------------------------------------------------------
def build_scan():
    nc = bacc.Bacc("TRN2", target_bir_lowering=False, debug=False,
                   enable_asserts=False, num_devices=NCORES)
    # partials reordered host-side to [NCORES, T, B, GATE]
    parts_d = nc.dram_tensor("parts", [NCORES, B * T * GATE], F32, kind="ExternalInput")
    biasg_d = nc.dram_tensor("biasg", [B, T * GATE], F32, kind="ExternalInput")
    whhT_d = nc.dram_tensor("whhT", [HID, GATE], F32, kind="ExternalInput")
    fc1w_d = nc.dram_tensor("fc1w", [HID, 512], F32, kind="ExternalInput")
    fc1b_d = nc.dram_tensor("fc1b", [B, 512], F32, kind="ExternalInput")
    fc2w_d = nc.dram_tensor("fc2w", [512, 1], F32, kind="ExternalInput")
    fc2b_d = nc.dram_tensor("fc2b", [B, 1], F32, kind="ExternalInput")
    out_d = nc.dram_tensor("out", [B, 1], F32, kind="ExternalOutput")
    with tile.TileContext(nc) as tc:
        with (
            tc.tile_pool(name="sb", bufs=1) as sp,
            tc.tile_pool(name="wk", bufs=2) as wk,
            tc.tile_pool(name="ps", bufs=1, space="PSUM") as pp,
            tc.tile_pool(name="dramc", bufs=1, space="DRAM") as dpc,
        ):
            ident = sp.tile([P, P], F32)
            make_identity(nc, ident[:])
            QW = B * T * GATE // P     # 320
            parts_t = sp.tile([P, QW, NCORES], F32)
            nc.sync.dma_start(out=parts_t[:],
                              in_=parts_d[:, :].rearrange("r (p q) -> p q r", p=P))
            s_t = sp.tile([P, QW], F32)
            nc.vector.tensor_reduce(out=s_t[:], in_=parts_t[:],
                                    axis=mybir.AxisListType.X, op=OP.add)
            gsc_d = dpc.tile([B, T * GATE], F32)
            nc.sync.dma_start(out=gsc_d[:].rearrange("b (k q) -> (b k) q", q=QW),
                              in_=s_t[:])
            gih_t = sp.tile([B, T * GATE], F32)
            nc.sync.dma_start(out=gih_t[:], in_=gsc_d[:, :])
            biasg_t = sp.tile([B, T * GATE], F32)
            nc.sync.dma_start(out=biasg_t[:], in_=biasg_d[:, :])
            nc.vector.tensor_tensor(out=gih_t[:], in0=gih_t[:], in1=biasg_t[:],
                                    op=OP.add)
            whhT_t = sp.tile([P, 2, GATE], F32)
            nc.sync.dma_start(out=whhT_t[:],
                              in_=whhT_d[:, :].rearrange("(k p) q -> p k q", p=P))
            fc1w_t = sp.tile([P, 2, 512], F32)
            nc.sync.dma_start(out=fc1w_t[:],
                              in_=fc1w_d[:, :].rearrange("(k p) q -> p k q", p=P))
            fc1b_t = sp.tile([B, 512], F32)
            nc.sync.dma_start(out=fc1b_t[:], in_=fc1b_d[:, :])
            fc2w_t = sp.tile([P, 4, 1], F32)
            nc.sync.dma_start(out=fc2w_t[:],
                              in_=fc2w_d[:, :].rearrange("(k p) q -> p k q", p=P))
            fc2b_t = sp.tile([B, 1], F32)
            nc.sync.dma_start(out=fc2b_t[:], in_=fc2b_d[:, :])

            h_t = sp.tile([B, HID], F32, tag="h")
            c_t = sp.tile([B, HID], F32, tag="c")
            nc.vector.memset(h_t[:], 0.0)
            nc.vector.memset(c_t[:], 0.0)
            hT_t = sp.tile([P, 2, B], F32, tag="hT")
            nc.vector.memset(hT_t[:], 0.0)

            def transpose_to(dst3, src, nk):
                # src [B, nk*128] -> dst3 [128, nk, B]
                for k in range(nk):
                    tps = pp.tile([P, B], F32, tag="tps")
                    nc.tensor.transpose(out=tps[:, :],
                                        in_=src[:, k * P:(k + 1) * P],
                                        identity=ident[:B, :B])
                    nc.vector.tensor_copy(out=dst3[:, k, :], in_=tps[:, :])

            for t in range(T):
                gps = pp.tile([B, GATE], F32, tag="gps")
                for k in range(2):
                    for s0 in range(0, GATE, 512):
                        nc.tensor.matmul(out=gps[:, s0:s0 + 512],
                                         lhsT=hT_t[:, k, :],
                                         rhs=whhT_t[:, k, s0:s0 + 512],
                                         start=(k == 0), stop=(k == 1))
                g_t = wk.tile([B, GATE], F32, tag="g")
                nc.vector.tensor_tensor(out=g_t[:], in0=gps[:, :],
                                        in1=gih_t[:, t * GATE:(t + 1) * GATE], op=OP.add)
                si = wk.tile([B, HID], F32, tag="si")
                sf = wk.tile([B, HID], F32, tag="sf")
                sg = wk.tile([B, HID], F32, tag="sg")
                so = wk.tile([B, HID], F32, tag="so")
                nc.scalar.activation(out=si[:], in_=g_t[:, 0:HID], func=AF.Sigmoid)
                nc.scalar.activation(out=sf[:], in_=g_t[:, HID:2 * HID], func=AF.Sigmoid)
                nc.scalar.activation(out=sg[:], in_=g_t[:, 2 * HID:3 * HID], func=AF.Tanh)
                nc.scalar.activation(out=so[:], in_=g_t[:, 3 * HID:4 * HID], func=AF.Sigmoid)
                c_new = sp.tile([B, HID], F32, tag=f"c{t}", name=f"c{t}")
                nc.vector.tensor_tensor(out=c_new[:], in0=sf[:], in1=c_t[:], op=OP.mult)
                nc.vector.tensor_tensor(out=si[:], in0=si[:], in1=sg[:], op=OP.mult)
                nc.vector.tensor_tensor(out=c_new[:], in0=c_new[:], in1=si[:], op=OP.add)
                tc_t = wk.tile([B, HID], F32, tag="tc")
                nc.scalar.activation(out=tc_t[:], in_=c_new[:], func=AF.Tanh)
                h_new = sp.tile([B, HID], F32, tag=f"h{t}", name=f"h{t}")
                nc.vector.tensor_tensor(out=h_new[:], in0=so[:], in1=tc_t[:], op=OP.mult)
                c_t = c_new
                h_t = h_new
                if t < T - 1:
                    hT_t = sp.tile([P, 2, B], F32, tag=f"hT{t}", name=f"hT{t}")
                    transpose_to(hT_t, h_new[:], 2)

            last_t = wk.tile([B, HID], F32, tag="last")
            nc.scalar.activation(out=last_t[:], in_=h_t[:], func=AF.Relu)
            lastT = sp.tile([P, 2, B], F32, tag="lastT")
            transpose_to(lastT, last_t[:], 2)
            hps = pp.tile([B, 512], F32, tag="hps")
            for k in range(2):
                nc.tensor.matmul(out=hps[:, :], lhsT=lastT[:, k, :],
                                 rhs=fc1w_t[:, k, :], start=(k == 0), stop=(k == 1))
            hid_t = sp.tile([B, 512], F32, tag="hid")
            nc.vector.tensor_tensor(out=hid_t[:], in0=hps[:, :], in1=fc1b_t[:], op=OP.add)
            nc.scalar.activation(out=hid_t[:], in_=hid_t[:], func=AF.Relu)
            hidT = sp.tile([P, 4, B], F32, tag="hidT")
            transpose_to(hidT, hid_t[:], 4)
            ops = pp.tile([B, 1], F32, tag="ops")
            for k in range(4):
                nc.tensor.matmul(out=ops[:, :], lhsT=hidT[:, k, :],
                                 rhs=fc2w_t[:, k, :], start=(k == 0), stop=(k == 3))
            o_t = wk.tile([B, 1], F32, tag="o")
            nc.vector.tensor_tensor(out=o_t[:], in0=ops[:, :], in1=fc2b_t[:], op=OP.add)
            nc.sync.dma_start(out=out_d[:, :], in_=o_t[:])
    nc.compile()
    return nc


# ----------------------------------------------------------------------------
# kernel entry
# ----------------------------------------------------------------------------
def kernel(**inputs):
    x = np.asarray(inputs["x"], np.float32)
    edge_index = np.asarray(inputs["edge_index"])
    edge_attr = np.asarray(inputs["edge_attr"], np.float32)

    gp = prep_graph(edge_index, edge_attr, inputs)
    key = tuple(gp["chunks_per_blk"])
    if ("A", key) not in _cache:
        _cache[("A", key)] = build_gat(gp["chunks_per_blk"])
    if "B" not in _cache:
        _cache["B"] = build_gates()
    if "C" not in _cache:
        _cache["C"] = build_scan()
    ncA, ncB, ncC = _cache[("A", key)], _cache["B"], _cache["C"]

    # ---- Launch A inputs ----
    xg = x.reshape(G, N, 8)                            # graph-major (b*T+t)
    xT_pad = np.zeros((G, 8, NPAD), np.float32)
    xT_pad[:, :, :N] = xg.transpose(0, 2, 1)
    w01 = np.concatenate([inputs["w_l0"], inputs["w_r0"]], 1).astype(np.float32)
    w11 = np.concatenate([inputs["w_l1"], inputs["w_r1"]], 1).astype(np.float32)
    w21 = np.concatenate([inputs["w_l2"], inputs["w_r2"]], 1).astype(np.float32)
    atts = [inputs["att0"], inputs["att1"], inputs["att2"]]
    biases = [inputs["b0"], inputs["b1"], inputs["b2"]]
    common = {
        "w01": w01, "w11": w11, "w21": w21,
        "pen": gp["pen"], "pne": gp["pne"], "idx": gp["idx"],
    }
    for l in range(3):
        common[f"ee{l}"] = gp["ees"][l]
        common[f"attb{l}"] = _bcast_const(atts[l], GL)
        common[f"biasb{l}"] = _bcast_const(biases[l], GL)
    in_maps = []
    for c in range(NCORES):
        m = dict(common)
        m["xT"] = np.ascontiguousarray(xT_pad[c * GL:(c + 1) * GL])  # [GL, 8, NPAD]
        in_maps.append(m)
    resA = bass_utils.run_bass_kernel_spmd(ncA, in_maps, core_ids=list(range(NCORES)))
    emb_all = np.concatenate([resA.results[c]["emb"] for c in range(NCORES)], 0)

    # ---- Launch B ----
    embT = np.zeros((NCORES, KPAD, G), np.float32)
    embT_full = emb_all.T                              # [64000, 40]
    wT_full = np.asarray(inputs["w_ih"], np.float32).T  # [64000, 1024]
    wT = np.zeros((NCORES, KPAD, GATE), np.float32)
    for c in range(NCORES):
        embT[c, :KSL] = embT_full[c * KSL:(c + 1) * KSL]
        wT[c, :KSL] = wT_full[c * KSL:(c + 1) * KSL]
    in_mapsB = [{"embT": embT[c], "wT": wT[c]} for c in range(NCORES)]
    resB = bass_utils.run_bass_kernel_spmd(ncB, in_mapsB, core_ids=list(range(NCORES)))
    parts = np.stack([resB.results[c]["part"] for c in range(NCORES)], 0)  # [8, 40, 1024]

    # ---- Launch C ----
    # graphs are (b*T+t): group per batch row, time along free dim
    parts_tb = parts.reshape(NCORES, B * T * GATE)
    biasg = np.broadcast_to(
        (np.asarray(inputs["b_ih"], np.float32)
         + np.asarray(inputs["b_hh"], np.float32)), (B, T, GATE)).reshape(
        B, T * GATE).copy()
    in_mapsC = [{
        "parts": parts_tb,
        "biasg": biasg,
        "whhT": np.asarray(inputs["w_hh"], np.float32).T.copy(),
        "fc1w": np.asarray(inputs["fc1_w"], np.float32),
        "fc1b": np.broadcast_to(np.asarray(inputs["fc1_b"], np.float32), (B, 512)).copy(),
        "fc2w": np.asarray(inputs["fc2_w"], np.float32),
        "fc2b": np.broadcast_to(np.asarray(inputs["fc2_b"], np.float32), (B, 1)).copy(),
    } for _ in range(NCORES)]
    resC = bass_utils.run_bass_kernel_spmd(ncC, in_mapsC, core_ids=list(range(NCORES)))
    return resC.results[0]["out"].astype(np.float32)



# revision 10
# speedup vs baseline: 2.1467x; 2.1467x over previous
"""Trainium2 Bass kernel for nn_GAT_LSTM: 3-layer GATv2 stack + LSTM + FC head.

Sharding (8 NeuronCores):
  Launch A: data-parallel over the 40 (B,T) graphs -> 5 graphs/core. GATv2
            edges sorted by dst into 128-node blocks; per-chunk one-hot
            matmuls (bf16) do xr[dst] gather + scatter on the PE; xl[src]
            comes from bf16 indirect-DMA gathers; the v = xl+xr+ee sum is
            assembled entirely in PSUM by the PE (identity matmuls, ee via
            a stride-0 broadcast rhs). Act engine runs Lrelu/Exp in per-block
            batches (2 act-table loads per block instead of 2 per chunk).
  Launch B: LSTM w_ih [1024,64000] column-sharded 8-way (bf16).
  Launch C: partial-gate reduce + LSTM scan + FC head.
Host work between launches is indexing/reshaping/dtype-casting only.
"""
import sys

for _p in ("/opt/trn_rl_repo", "/root/.axon_site/_ro/trn_rl_repo"):
    if _p not in sys.path:
        sys.path.insert(0, _p)

import ml_dtypes
import numpy as np

import concourse.bass as bass
import concourse.bacc as bacc
import concourse.mybir as mybir
import concourse.tile as tile
from concourse import bass_utils
from concourse.masks import make_identity

F32 = mybir.dt.float32
BF16 = mybir.dt.bfloat16
I32 = mybir.dt.int32
AF = mybir.ActivationFunctionType
OP = mybir.AluOpType
NPBF = ml_dtypes.bfloat16

P = 128
N = 2000
NPAD = 2048
NBLK = 16
B, T = 4, 10
G = B * T          # 40 graphs
NCORES = 8
GL = G // NCORES   # 5 graphs per core
E0 = 16000
EFULL = E0 + N     # with self loops
LAYERS = [(8, 4, 32), (128, 4, 32), (128, 4, 8)]   # (F_in, H, C)
HID = 256
GATE = 4 * HID     # 1024
EMB = N * 32       # 64000
KSL = EMB // NCORES                 # 8000 w_ih rows per core
KPAD = ((KSL + 127) // 128) * 128   # 8064
QW = G * GATE // P                  # 320

_cache = {}


# ----------------------------------------------------------------------------
# host-side graph preprocessing (indexing + tiny edge-static math)
# ----------------------------------------------------------------------------
def prep_graph(edge_index, edge_attr, weights):
    src = np.concatenate([np.asarray(edge_index[0], np.int64),
                          np.arange(N, dtype=np.int64)])
    dst = np.concatenate([np.asarray(edge_index[1], np.int64),
                          np.arange(N, dtype=np.int64)])
    ea = np.concatenate(
        [edge_attr, np.broadcast_to(np.asarray(edge_attr).mean(0), (N, 2))], axis=0
    ).astype(np.float32)

    order = np.argsort(dst, kind="stable")
    src_s, dst_s, ea_s = src[order], dst[order], ea[order]

    blk_of = dst_s // P
    chunks_per_blk = []
    src_pad, ldst_pad, valid_pad, ea_pad = [], [], [], []
    for b in range(NBLK):
        sel = blk_of == b
        e_src = src_s[sel]
        e_ldst = dst_s[sel] - b * P
        e_ea = ea_s[sel]
        ne = len(e_src)
        nch = max(1, (ne + P - 1) // P)
        pad = nch * P - ne
        src_pad.append(np.concatenate([e_src, np.zeros(pad, np.int64)]))
        ldst_pad.append(np.concatenate([e_ldst, np.zeros(pad, np.int64)]))
        valid_pad.append(np.concatenate([np.ones(ne, bool), np.zeros(pad, bool)]))
        ea_pad.append(np.concatenate([e_ea, np.zeros((pad, 2), np.float32)]))
        chunks_per_blk.append(nch)

    src_all = np.concatenate(src_pad)
    ldst_all = np.concatenate(ldst_pad)
    valid_all = np.concatenate(valid_pad)
    ea_all = np.concatenate(ea_pad)
    nch_total = sum(chunks_per_blk)

    idx = src_all.reshape(nch_total, P).T.astype(np.int32).copy()      # [128, NCH]
    pen = np.zeros((nch_total, P, P), np.float32)                      # [j, e, n]
    jj = np.repeat(np.arange(nch_total), P)
    ee_pos = np.tile(np.arange(P), nch_total)
    pen[jj[valid_all], ee_pos[valid_all], ldst_all[valid_all]] = 1.0

    # device layouts: pen_h [e, j, n], pne_h [n, j, e] (bf16)
    pen_h = np.ascontiguousarray(pen.transpose(1, 0, 2)).astype(NPBF)
    pne_h = np.ascontiguousarray(pen.transpose(2, 0, 1)).astype(NPBF)

    # per-layer edge-feature terms ee = ea @ We, device layout [e, j, fo] bf16
    ees = []
    for key in ("w_e0", "w_e1", "w_e2"):
        we = np.asarray(weights[key], np.float32)
        ee = (ea_all @ we).astype(np.float32)      # [NCH*128, fo]
        ee = ee.reshape(nch_total, P, -1).transpose(1, 0, 2)
        ees.append(np.ascontiguousarray(ee).astype(NPBF))
    return dict(
        chunks_per_blk=chunks_per_blk, nch_total=nch_total,
        idx=idx, pen_h=pen_h, pne_h=pne_h, ees=ees,
    )


def _bcast_const(vec, reps):
    """[F] -> [128, reps*F] partition-broadcast bf16 constant."""
    t = np.tile(np.asarray(vec, np.float32).reshape(-1), reps)
    return np.ascontiguousarray(np.broadcast_to(t, (P, t.size))).astype(NPBF)


# ----------------------------------------------------------------------------
# Launch A: GAT stack, 5 graphs per core
# ----------------------------------------------------------------------------
def build_gat(chunks_per_blk, debug_layers=(), num_devices=NCORES):
    nch_total = sum(chunks_per_blk)
    maxch = max(chunks_per_blk)
    nc = bacc.Bacc("TRN2", target_bir_lowering=False, debug=False,
                   enable_asserts=False, num_devices=num_devices)
    xT_d = nc.dram_tensor("xT", [8, GL, NPAD], BF16, kind="ExternalInput")
    w01_d = nc.dram_tensor("w01", [8, 256], BF16, kind="ExternalInput")
    w11_d = nc.dram_tensor("w11", [128, 256], BF16, kind="ExternalInput")
    w21_d = nc.dram_tensor("w21", [128, 64], BF16, kind="ExternalInput")
    ee_d = [nc.dram_tensor(f"ee{l}", [P, nch_total, LAYERS[l][1] * LAYERS[l][2]],
                           BF16, kind="ExternalInput") for l in range(3)]
    pen_d = nc.dram_tensor("pen", [P, nch_total, P], BF16, kind="ExternalInput")
    pne_d = nc.dram_tensor("pne", [P, nch_total, P], BF16, kind="ExternalInput")
    idx_d = nc.dram_tensor("idx", [P, nch_total], I32, kind="ExternalInput")
    attb_d = [nc.dram_tensor(f"attb{l}", [P, GL * LAYERS[l][1] * LAYERS[l][2]],
                             BF16, kind="ExternalInput") for l in range(3)]
    biasb_d = [nc.dram_tensor(f"biasb{l}", [P, GL * LAYERS[l][1] * LAYERS[l][2]],
                              BF16, kind="ExternalInput") for l in range(3)]
    emb_d = nc.dram_tensor("emb", [GL, EMB], BF16, kind="ExternalOutput")
    dbg_d = {l: nc.dram_tensor(f"dbg{l}", [NBLK, P, GL, 128], F32,
                               kind="ExternalOutput") for l in debug_layers}

    ws_d = [w01_d, w11_d, w21_d]

    with tile.TileContext(nc) as tc:
        with (
            tc.tile_pool(name="const", bufs=1) as cp,
            tc.tile_pool(name="xr", bufs=1) as xrp,
            tc.tile_pool(name="oblk", bufs=1) as obp,
            tc.tile_pool(name="edge", bufs=1) as ep,
            tc.tile_pool(name="work", bufs=3) as wp,
            tc.tile_pool(name="stage", bufs=2) as stg,
            tc.tile_pool(name="stream", bufs=2) as strm,
            tc.tile_pool(name="psv", bufs=2, space="PSUM") as pv,
            tc.tile_pool(name="psacc", bufs=1, space="PSUM") as pacc,
            tc.tile_pool(name="pstp", bufs=1, space="PSUM") as ptp,
            tc.tile_pool(name="dram", bufs=1, space="DRAM") as dp,
        ):
            ident = cp.tile([P, P], F32)
            make_identity(nc, ident[:])
            identb = cp.tile([P, P], BF16)
            nc.vector.tensor_copy(out=identb[:], in_=ident[:])
            idx_t = cp.tile([P, nch_total], I32)
            nc.sync.dma_start(out=idx_t[:], in_=idx_d[:, :])
            xT_t = cp.tile([8, GL, NPAD], BF16)
            nc.sync.dma_start(out=xT_t[:], in_=xT_d[:, :, :])
            w_t = [cp.tile([8, 256], BF16, tag="w0", name="w0t"),
                   cp.tile([128, 256], BF16, tag="w1", name="w1t"),
                   cp.tile([128, 64], BF16, tag="w2", name="w2t")]
            attb_t, biasb_t = [], []
            for l in range(3):
                nc.sync.dma_start(out=w_t[l][:], in_=ws_d[l][:, :])
                fo = LAYERS[l][1] * LAYERS[l][2]
                at = cp.tile([P, GL * fo], BF16, tag=f"attb{l}", name=f"attb{l}t")
                bt = cp.tile([P, GL * fo], BF16, tag=f"biasb{l}", name=f"biasb{l}t")
                nc.sync.dma_start(out=at[:], in_=attb_d[l][:, :])
                nc.sync.dma_start(out=bt[:], in_=biasb_d[l][:, :])
                attb_t.append(at)
                biasb_t.append(bt)

            xl_dram = dp.tile([NPAD, GL * 128], BF16)
            xl_dram2 = dp.tile([NPAD, GL * 32], BF16)

            for l, (fin, hh, cc) in enumerate(LAYERS):
                fo = hh * cc
                gfo = GL * fo
                ghh = GL * hh
                xld = xl_dram2 if l == 2 else xl_dram
                s512 = [(s, min(s + 512, gfo)) for s in range(0, gfo, 512)]

                # ---- transform: xl = h@Wl (to DRAM), xr = h@Wr (SBUF) ----
                xr_blocks = []
                o_prev = None if l == 0 else o_blocks
                for b in range(NBLK):
                    if l > 0:
                        # hT5 = transpose of the 5 per-graph [128n,128f] blocks
                        tps = ptp.tile([P, GL * fin], BF16, tag="tp")
                        for g in range(GL):
                            nc.tensor.transpose(
                                out=tps[:, g * fin:(g + 1) * fin],
                                in_=o_prev[b][:, g, :],
                                identity=identb[:])
                        hT5 = stg.tile([P, GL, fin], BF16, tag="hT5")
                        nc.vector.tensor_copy(out=hT5[:], in_=tps[:, :])
                    ps_xl = pv.tile([P, 640], F32, tag="v")
                    ps_xr = pv.tile([P, 640], F32, tag="v")
                    for g in range(GL):
                        if l == 0:
                            lhsT = xT_t[:8, g, b * P:(b + 1) * P]
                        else:
                            lhsT = hT5[:, g, :]
                        nc.tensor.matmul(out=ps_xl[:, g * fo:(g + 1) * fo],
                                         lhsT=lhsT, rhs=w_t[l][:fin, :fo],
                                         start=True, stop=True)
                        nc.tensor.matmul(out=ps_xr[:, g * fo:(g + 1) * fo],
                                         lhsT=lhsT, rhs=w_t[l][:fin, fo:2 * fo],
                                         start=True, stop=True)
                    xl_st = stg.tile([P, 640], BF16, tag="xl_st")
                    nc.vector.tensor_copy(out=xl_st[:, :gfo], in_=ps_xl[:, :gfo])
                    xr_b = xrp.tile([P, GL, 128], BF16, tag=f"xr{b}")
                    nc.scalar.activation(
                        out=xr_b[:].rearrange("p g f -> p (g f)")[:, :gfo],
                        in_=ps_xr[:, :gfo], func=AF.Copy)
                    nc.sync.dma_start(
                        out=xld[b * P:(b + 1) * P, :gfo], in_=xl_st[:, :gfo])
                    xr_blocks.append(xr_b)
                if l < 2:
                    o_blocks = [obp.tile([P, GL, fo], BF16, tag=f"o{l}_{b}",
                                         name=f"o{l}_{b}")
                                for b in range(NBLK)]

                # ---- edge phase ----
                j0 = 0
                for b in range(NBLK):
                    nch = chunks_per_blk[b]
                    pen_t = strm.tile([P, maxch, P], BF16, tag="pen")
                    nc.sync.dma_start(out=pen_t[:, :nch, :],
                                      in_=pen_d[:, j0:j0 + nch, :])
                    pne_t = strm.tile([P, maxch, P], BF16, tag="pne")
                    nc.sync.dma_start(out=pne_t[:, :nch, :],
                                      in_=pne_d[:, j0:j0 + nch, :])
                    ee_t = strm.tile([P, maxch, fo], BF16, tag="ee")
                    nc.sync.dma_start(out=ee_t[:, :nch, :],
                                      in_=ee_d[l][:, j0:j0 + nch, :])
                    ps_acc = pacc.tile([P, 640], F32, tag="acc")
                    ps_den = pacc.tile([P, 20], F32, tag="den")

                    xr_flat = xr_blocks[b][:].rearrange("p g f -> p (g f)")
                    g_ts, m_ts, lg_ts, p_ts = [], [], [], []
                    # S1: gather + PSUM assembly + lrelu
                    for j in range(nch):
                        jg = j0 + j
                        g_t = ep.tile([P, 640], BF16, tag=f"g{j}")
                        nc.gpsimd.indirect_dma_start(
                            out=g_t[:, :gfo], out_offset=None,
                            in_=xld[:, :],
                            in_offset=bass.IndirectOffsetOnAxis(
                                ap=idx_t[:, jg:jg + 1], axis=0))
                        ps_v = pv.tile([P, 640], F32, tag="v")
                        for (s0, s1) in s512:
                            nc.tensor.matmul(out=ps_v[:, s0:s1],
                                             lhsT=pne_t[:, j, :],
                                             rhs=xr_flat[:, s0:s1],
                                             start=True, stop=False)
                            nc.tensor.matmul(out=ps_v[:, s0:s1],
                                             lhsT=identb[:],
                                             rhs=g_t[:, s0:s1],
                                             start=False, stop=False)
                        eeb = ee_t[:, j:j + 1, :].to_broadcast([P, GL, fo])
                        if gfo == 640:
                            nc.tensor.matmul(out=ps_v[:, 0:512], lhsT=identb[:],
                                             rhs=eeb[:, 0:4, :],
                                             start=False, stop=True)
                            nc.tensor.matmul(out=ps_v[:, 512:640], lhsT=identb[:],
                                             rhs=eeb[:, 4:5, :],
                                             start=False, stop=True,
                                             skip_group_check=True)
                        else:
                            nc.tensor.matmul(out=ps_v[:, 0:gfo], lhsT=identb[:],
                                             rhs=eeb,
                                             start=False, stop=True)
                        m_t = ep.tile([P, 640], BF16, tag=f"m{j}")
                        nc.scalar.activation(out=m_t[:, :gfo], in_=ps_v[:, :gfo],
                                             func=AF.Prelu, alpha=0.2)
                        g_ts.append(g_t)
                        m_ts.append(m_t)
                    # S2: att-mult + grouped reduce -> logits
                    for j in range(nch):
                        am = wp.tile([P, 640], BF16, tag="am")
                        nc.vector.tensor_tensor(out=am[:, :gfo], in0=m_ts[j][:, :gfo],
                                                in1=attb_t[l][:], op=OP.mult)
                        lg = ep.tile([P, ghh], F32, tag=f"lg{j}")
                        nc.vector.tensor_reduce(
                            out=lg[:],
                            in_=am[:, :gfo].rearrange("p (t c) -> p t c", c=cc),
                            axis=mybir.AxisListType.X, op=OP.add)
                        lg_ts.append(lg)
                    # S3: exp (Act, one table load per block)
                    for j in range(nch):
                        p_t = ep.tile([P, ghh], BF16, tag=f"p{j}")
                        nc.scalar.activation(out=p_t[:], in_=lg_ts[j][:], func=AF.Exp)
                        p_ts.append(p_t)
                    # S4: pxl + den/scatter accumulation
                    for j in range(nch):
                        pxl = wp.tile([P, 640], BF16, tag="pxl")
                        nc.vector.tensor_tensor(
                            out=pxl[:, :gfo].rearrange("p (t c) -> p t c", c=cc),
                            in0=g_ts[j][:, :gfo].rearrange("p (t c) -> p t c", c=cc),
                            in1=p_ts[j][:].rearrange("p (t u) -> p t u", u=1)
                                .to_broadcast([P, ghh, cc]),
                            op=OP.mult)
                        nc.tensor.matmul(out=ps_den[:, :ghh],
                                         lhsT=pen_t[:, j, :], rhs=p_ts[j][:],
                                         start=(j == 0), stop=(j == nch - 1))
                        for (s0, s1) in s512:
                            nc.tensor.matmul(out=ps_acc[:, s0:s1],
                                             lhsT=pen_t[:, j, :],
                                             rhs=pxl[:, s0:s1],
                                             start=(j == 0), stop=(j == nch - 1))
                    # tail: out = relu(ps_acc/den + bias)
                    den_t = wp.tile([P, ghh], F32, tag="den")
                    nc.vector.tensor_scalar_add(out=den_t[:],
                                                in0=ps_den[:, :ghh],
                                                scalar1=1e-16)
                    rec_t = wp.tile([P, ghh], F32, tag="rec")
                    nc.vector.reciprocal(out=rec_t[:], in_=den_t[:])
                    o_t = wp.tile([P, 640], BF16, tag="o_t")
                    nc.vector.tensor_tensor(
                        out=o_t[:, :gfo].rearrange("p (t c) -> p t c", c=cc),
                        in0=ps_acc[:, :gfo].rearrange("p (t c) -> p t c", c=cc),
                        in1=rec_t[:].rearrange("p (t u) -> p t u", u=1)
                            .to_broadcast([P, ghh, cc]),
                        op=OP.mult)
                    nc.vector.tensor_tensor(out=o_t[:, :gfo], in0=o_t[:, :gfo],
                                            in1=biasb_t[l][:], op=OP.add)
                    if l < 2:
                        nc.scalar.activation(
                            out=o_blocks[b][:].rearrange("p g f -> p (g f)"),
                            in_=o_t[:, :gfo], func=AF.Relu)
                    else:
                        o2 = stg.tile([P, GL, 32], BF16, tag="o2")
                        nc.scalar.activation(
                            out=o2[:].rearrange("p g f -> p (g f)"),
                            in_=o_t[:, :gfo], func=AF.Relu)
                        rows = min(P, N - b * P)
                        nc.sync.dma_start(
                            out=emb_d[:, :].rearrange("g (n c) -> g n c", c=32)[
                                :, b * P:b * P + rows, :].rearrange("g p c -> p g c"),
                            in_=o2[:rows, :, :])
                    j0 += nch
                if l in debug_layers:
                    for b in range(NBLK):
                        dbf = stg.tile([P, GL, 128], F32, tag="dbf", name="dbf")
                        nc.vector.memset(dbf[:], 0.0)
                        nc.vector.tensor_copy(
                            out=dbf[:].rearrange("p g f -> p (g f)")[:, :gfo],
                            in_=o_blocks[b][:].rearrange("p g f -> p (g f)"))
                        nc.sync.dma_start(out=dbg_d[l][b, :, :, :], in_=dbf[:])
    nc.compile()
    return nc


# ----------------------------------------------------------------------------
# Launch B: partial LSTM input-gate products (w_ih column shard, bf16)
# ----------------------------------------------------------------------------
def build_gates():
    nc = bacc.Bacc("TRN2", target_bir_lowering=False, debug=False,
                   enable_asserts=False, num_devices=NCORES)
    embT_d = nc.dram_tensor("embT", [KPAD, G], BF16, kind="ExternalInput")
    wT_d = nc.dram_tensor("wT", [KPAD, GATE], BF16, kind="ExternalInput")
    part_d = nc.dram_tensor("part", [G, GATE], F32, kind="ExternalOutput")
    KCH = KPAD // P
    with tile.TileContext(nc) as tc:
        with (
            tc.tile_pool(name="sb", bufs=1) as sp,
            tc.tile_pool(name="wstream", bufs=4) as wsp,
            tc.tile_pool(name="ps", bufs=1, space="PSUM") as pp,
        ):
            embT_t = sp.tile([P, KCH, G], BF16)
            nc.sync.dma_start(out=embT_t[:],
                              in_=embT_d[:, :].rearrange("(k p) g -> p k g", p=P))
            ps = pp.tile([G, GATE], F32)
            for k in range(KCH):
                w_t = wsp.tile([P, GATE], BF16, tag="w")
                nc.sync.dma_start(out=w_t[:], in_=wT_d[k * P:(k + 1) * P, :])
                for s0 in range(0, GATE, 512):
                    nc.tensor.matmul(out=ps[:, s0:s0 + 512],
                                     lhsT=embT_t[:, k, :],
                                     rhs=w_t[:, s0:s0 + 512],
                                     start=(k == 0), stop=(k == KCH - 1))
            out_t = sp.tile([G, GATE], F32)
            nc.vector.tensor_copy(out=out_t[:], in_=ps[:, :])
            nc.sync.dma_start(out=part_d[:, :], in_=out_t[:])
    nc.compile()
    return nc


# ----------------------------------------------------------------------------
# Launch C: reduce partials + LSTM scan + FC head
# ----------------------------------------------------------------------------
def build_scan():
    nc = bacc.Bacc("TRN2", target_bir_lowering=False, debug=False,
                   enable_asserts=False, num_devices=NCORES)
    # partials pre-laid-out host-side to [128, QW, NCORES] (contiguous)
    parts_d = nc.dram_tensor("parts", [P, QW * NCORES], F32, kind="ExternalInput")
    biasg_d = nc.dram_tensor("biasg", [B, T * GATE], F32, kind="ExternalInput")
    whhT_d = nc.dram_tensor("whhT", [HID, GATE], BF16, kind="ExternalInput")
    fc1w_d = nc.dram_tensor("fc1w", [HID, 512], BF16, kind="ExternalInput")
    fc1b_d = nc.dram_tensor("fc1b", [B, 512], F32, kind="ExternalInput")
    fc2w_d = nc.dram_tensor("fc2w", [512, 1], BF16, kind="ExternalInput")
    fc2b_d = nc.dram_tensor("fc2b", [B, 1], F32, kind="ExternalInput")
    out_d = nc.dram_tensor("out", [B, 1], F32, kind="ExternalOutput")
    with tile.TileContext(nc) as tc:
        with (
            tc.tile_pool(name="sb", bufs=1) as sp,
            tc.tile_pool(name="wk", bufs=2) as wk,
            tc.tile_pool(name="ps", bufs=1, space="PSUM") as pp,
            tc.tile_pool(name="dramc", bufs=1, space="DRAM") as dpc,
        ):
            ident = sp.tile([P, P], F32)
            make_identity(nc, ident[:])
            parts_t = sp.tile([P, QW, NCORES], F32)
            nc.sync.dma_start(out=parts_t[:],
                              in_=parts_d[:, :].rearrange("p (q r) -> p q r", r=NCORES))
            s_t = sp.tile([P, QW], F32)
            nc.vector.tensor_reduce(out=s_t[:], in_=parts_t[:],
                                    axis=mybir.AxisListType.X, op=OP.add)
            gsc_d = dpc.tile([B, T * GATE], F32)
            nc.sync.dma_start(out=gsc_d[:].rearrange("b (k q) -> (b k) q", q=QW),
                              in_=s_t[:])
            gih_t = sp.tile([B, T * GATE], F32)
            nc.sync.dma_start(out=gih_t[:], in_=gsc_d[:, :])
            biasg_t = sp.tile([B, T * GATE], F32)
            nc.sync.dma_start(out=biasg_t[:], in_=biasg_d[:, :])
            nc.vector.tensor_tensor(out=gih_t[:], in0=gih_t[:], in1=biasg_t[:],
                                    op=OP.add)
            whhT_t = sp.tile([P, 2, GATE], BF16)
            nc.sync.dma_start(out=whhT_t[:],
                              in_=whhT_d[:, :].rearrange("(k p) q -> p k q", p=P))
            fc1w_t = sp.tile([P, 2, 512], BF16)
            nc.sync.dma_start(out=fc1w_t[:],
                              in_=fc1w_d[:, :].rearrange("(k p) q -> p k q", p=P))
            fc1b_t = sp.tile([B, 512], F32)
            nc.sync.dma_start(out=fc1b_t[:], in_=fc1b_d[:, :])
            fc2w_t = sp.tile([P, 4, 1], BF16)
            nc.sync.dma_start(out=fc2w_t[:],
                              in_=fc2w_d[:, :].rearrange("(k p) q -> p k q", p=P))
            fc2b_t = sp.tile([B, 1], F32)
            nc.sync.dma_start(out=fc2b_t[:], in_=fc2b_d[:, :])

            h_t = sp.tile([B, HID], F32, tag="h")
            c_t = sp.tile([B, HID], F32, tag="c")
            nc.vector.memset(h_t[:], 0.0)
            nc.vector.memset(c_t[:], 0.0)
            hT_t = sp.tile([P, 2, B], BF16, tag="hT")
            nc.vector.memset(hT_t[:], 0.0)

            def transpose_to(dst3, src, nk):
                # src [B, nk*128] f32 -> dst3 [128, nk, B] bf16
                for k in range(nk):
                    tps = pp.tile([P, B], F32, tag="tps")
                    nc.tensor.transpose(out=tps[:, :],
                                        in_=src[:, k * P:(k + 1) * P],
                                        identity=ident[:B, :B])
                    nc.vector.tensor_copy(out=dst3[:, k, :], in_=tps[:, :])

            for t in range(T):
                gps = pp.tile([B, GATE], F32, tag="gps")
                for k in range(2):
                    for s0 in range(0, GATE, 512):
                        nc.tensor.matmul(out=gps[:, s0:s0 + 512],
                                         lhsT=hT_t[:, k, :],
                                         rhs=whhT_t[:, k, s0:s0 + 512],
                                         start=(k == 0), stop=(k == 1))
                g_t = wk.tile([B, GATE], F32, tag="g")
                nc.vector.tensor_tensor(out=g_t[:], in0=gps[:, :],
                                        in1=gih_t[:, t * GATE:(t + 1) * GATE], op=OP.add)
                si = wk.tile([B, HID], F32, tag="si")
                sfo = wk.tile([B, 2 * HID], F32, tag="sfo")
                sg = wk.tile([B, HID], F32, tag="sg")
                nc.scalar.activation(out=si[:], in_=g_t[:, 0:HID], func=AF.Sigmoid)
                nc.scalar.activation(out=sfo[:, :HID], in_=g_t[:, HID:2 * HID],
                                     func=AF.Sigmoid)
                nc.scalar.activation(out=sfo[:, HID:], in_=g_t[:, 3 * HID:4 * HID],
                                     func=AF.Sigmoid)
                nc.scalar.activation(out=sg[:], in_=g_t[:, 2 * HID:3 * HID],
                                     func=AF.Tanh)
                c_new = sp.tile([B, HID], F32, tag=f"c{t}")
                nc.vector.tensor_tensor(out=c_new[:], in0=sfo[:, :HID], in1=c_t[:],
                                        op=OP.mult)
                nc.vector.tensor_tensor(out=si[:], in0=si[:], in1=sg[:], op=OP.mult)
                nc.vector.tensor_tensor(out=c_new[:], in0=c_new[:], in1=si[:],
                                        op=OP.add)
                tc_t = wk.tile([B, HID], F32, tag="tc")
                nc.scalar.activation(out=tc_t[:], in_=c_new[:], func=AF.Tanh)
                h_new = sp.tile([B, HID], F32, tag=f"h{t}")
                nc.vector.tensor_tensor(out=h_new[:], in0=sfo[:, HID:], in1=tc_t[:],
                                        op=OP.mult)
                c_t = c_new
                h_t = h_new
                if t < T - 1:
                    hT_t = sp.tile([P, 2, B], BF16, tag=f"hT{t}")
                    transpose_to(hT_t, h_new[:], 2)

            last_t = wk.tile([B, HID], F32, tag="last")
            nc.scalar.activation(out=last_t[:], in_=h_t[:], func=AF.Relu)
            lastT = sp.tile([P, 2, B], BF16, tag="lastT")
            transpose_to(lastT, last_t[:], 2)
            hps = pp.tile([B, 512], F32, tag="hps")
            for k in range(2):
                nc.tensor.matmul(out=hps[:, :], lhsT=lastT[:, k, :],
                                 rhs=fc1w_t[:, k, :], start=(k == 0), stop=(k == 1))
            hid_t = sp.tile([B, 512], F32, tag="hid")
            nc.vector.tensor_tensor(out=hid_t[:], in0=hps[:, :], in1=fc1b_t[:],
                                    op=OP.add)
            nc.scalar.activation(out=hid_t[:], in_=hid_t[:], func=AF.Relu)
            hidT = sp.tile([P, 4, B], BF16, tag="hidT")
            transpose_to(hidT, hid_t[:], 4)
            ops = pp.tile([B, 1], F32, tag="ops")
            for k in range(4):
                nc.tensor.matmul(out=ops[:, :], lhsT=hidT[:, k, :],
                                 rhs=fc2w_t[:, k, :], start=(k == 0), stop=(k == 3))
            o_t = wk.tile([B, 1], F32, tag="o")
            nc.vector.tensor_tensor(out=o_t[:], in0=ops[:, :], in1=fc2b_t[:],
                                    op=OP.add)
            nc.sync.dma_start(out=out_d[:, :], in_=o_t[:])
    nc.compile()
    return nc


# ----------------------------------------------------------------------------
# kernel entry
# ----------------------------------------------------------------------------
def kernel(**inputs):
    x = np.asarray(inputs["x"], np.float32)
    edge_index = np.asarray(inputs["edge_index"])
    edge_attr = np.asarray(inputs["edge_attr"], np.float32)

    gp = prep_graph(edge_index, edge_attr, inputs)
    key = tuple(gp["chunks_per_blk"])
    if ("A", key) not in _cache:
        _cache[("A", key)] = build_gat(gp["chunks_per_blk"])
    if "B" not in _cache:
        _cache["B"] = build_gates()
    if "C" not in _cache:
        _cache["C"] = build_scan()
    ncA, ncB, ncC = _cache[("A", key)], _cache["B"], _cache["C"]

    # ---- Launch A inputs ----
    xg = x.reshape(G, N, 8)
    w01 = np.concatenate([inputs["w_l0"], inputs["w_r0"]], 1).astype(NPBF)
    w11 = np.concatenate([inputs["w_l1"], inputs["w_r1"]], 1).astype(NPBF)
    w21 = np.concatenate([inputs["w_l2"], inputs["w_r2"]], 1).astype(NPBF)
    atts = [inputs["att0"], inputs["att1"], inputs["att2"]]
    biases = [inputs["b0"], inputs["b1"], inputs["b2"]]
    common = {
        "w01": w01, "w11": w11, "w21": w21,
        "pen": gp["pen_h"], "pne": gp["pne_h"], "idx": gp["idx"],
    }
    for l in range(3):
        common[f"ee{l}"] = gp["ees"][l]
        common[f"attb{l}"] = _bcast_const(atts[l], GL)
        common[f"biasb{l}"] = _bcast_const(biases[l], GL)
    in_maps = []
    for c in range(NCORES):
        m = dict(common)
        xc = np.zeros((8, GL, NPAD), np.float32)
        xc[:, :, :N] = xg[c * GL:(c + 1) * GL].transpose(2, 0, 1)
        m["xT"] = xc.astype(NPBF)
        in_maps.append(m)
    resA = bass_utils.run_bass_kernel_spmd(ncA, in_maps, core_ids=list(range(NCORES)))
    emb_all = np.concatenate(
        [np.asarray(resA.results[c]["emb"]) for c in range(NCORES)], 0)  # bf16 [G, EMB]

    # ---- Launch B ----
    embT_full = np.ascontiguousarray(emb_all.T)          # [64000, 40] bf16
    wT_full = np.asarray(inputs["w_ih"], np.float32).T.astype(NPBF)  # [64000, 1024]
    in_mapsB = []
    for c in range(NCORES):
        embT = np.zeros((KPAD, G), NPBF)
        wT = np.zeros((KPAD, GATE), NPBF)
        embT[:KSL] = embT_full[c * KSL:(c + 1) * KSL]
        wT[:KSL] = wT_full[c * KSL:(c + 1) * KSL]
        in_mapsB.append({"embT": embT, "wT": wT})
    resB = bass_utils.run_bass_kernel_spmd(ncB, in_mapsB, core_ids=list(range(NCORES)))
    parts = np.stack([np.asarray(resB.results[c]["part"], np.float32)
                      for c in range(NCORES)], 0)        # [8, 40, 1024]

    # ---- Launch C ----
    # partition p = (b, k); free = (q, r): flat gate idx = b*10240 + k*320 + q
    pr = parts.reshape(NCORES, B, 32, QW)                # [r, b, k, q]
    parts_pre = np.ascontiguousarray(
        pr.transpose(1, 2, 3, 0).reshape(P, QW * NCORES)).astype(np.float32)
    biasg = np.broadcast_to(
        (np.asarray(inputs["b_ih"], np.float32)
         + np.asarray(inputs["b_hh"], np.float32)), (B, T, GATE)).reshape(
        B, T * GATE).copy()
    in_mapsC = [{
        "parts": parts_pre,
        "biasg": biasg,
        "whhT": np.asarray(inputs["w_hh"], np.float32).T.copy().astype(NPBF),
        "fc1w": np.asarray(inputs["fc1_w"], np.float32).astype(NPBF),
        "fc1b": np.broadcast_to(np.asarray(inputs["fc1_b"], np.float32), (B, 512)).copy(),
        "fc2w": np.asarray(inputs["fc2_w"], np.float32).astype(NPBF),
        "fc2b": np.broadcast_to(np.asarray(inputs["fc2_b"], np.float32), (B, 1)).copy(),
    } for _ in range(NCORES)]
    resC = bass_utils.run_bass_kernel_spmd(ncC, in_mapsC, core_ids=list(range(NCORES)))
    return np.asarray(resC.results[0]["out"], np.float32)
